# revision 45
# baseline (speedup 1.0000x reference)
"""Trainium2 Bass kernel for nn_MemTransformerLM (Transformer-XL layer).

Sharding (8 NeuronCores): tensor-parallel over heads -- each core owns 2 of the
16 heads for all 4 batches, computes q/k/v/r projections for its heads, full
causal rel-attention for its (batch, head) pairs, and the column-slice of the
o-projection.  Per-batch partial attention outputs are summed across cores with
a ReduceScatter (bf16) that also hands each core a 256-token slice of every
batch; LayerNorm1 + position-wise FFN + LayerNorm2 then run token-parallel on
those rows with no further communication.  The host reassembles the full
[qlen, bsz, d_model] output from the per-core row slabs.

Key structure (vs the v1 baseline, 1885us -> ~1460us):
 - all activation streams bf16 (wT/rT shipped bf16, halving input DMA)
 - q+r_w_bias / q+r_r_bias / k+bk are folded into the projection-PSUM
   evictions in phase A (replaces 234us of per-batch gpsimd broadcast adds)
 - the rel-shift pad is NEG_BIG-filled once per buffer per batch, so causal
   masking falls out of the skew read -- no affine_select, no per-I memset
 - the skew is a per-head SWDGE accumulate-DMA that adds shifted BD directly
   into the AC score tile (no separate DVE add pass); exp(h0) overlaps the
   h1 skew
 - score matmuls for the core's two heads run concurrently in the PE array
   via row tiling (K=64 each)
 - NO DMA transposes anywhere: prob/av/v transposes run on the PE (the
   runtime serializes every DMA transpose against in-flight collectives --
   the comm-init collective and each ReduceScatter would stall them 40-70us)
 - ReduceScatter overlaps the next batch's attention; LayerNorm1 is floored
   past the attention region with tile_wait_until so the scheduler cannot
   backfill its RS-dependent ops into attention-era queue slots (their sem
   waits head-of-line-block the vector/sync queues)
"""

import sys

for _p in ("/opt/trn_rl_repo", "/root/.axon_site/_ro/trn_rl_repo"):
    if _p not in sys.path:
        sys.path.insert(0, _p)

import numpy as np

import concourse.bass as bass
import concourse.mybir as mybir
import concourse.tile as tile
from concourse import bacc
from concourse.masks import make_identity

F32 = mybir.dt.float32
BF16 = mybir.dt.bfloat16
AF = mybir.ActivationFunctionType
ALU = mybir.AluOpType

NEG_BIG = -30000.0


def build_program(qlen=2048, bsz=4, d_model=1024, d_head=64, heads_per_core=2,
                  d_inner=4096, n_cores=8, repeat=1, row_tile=True,
                  skew_accum=True):
    P = 128
    hpc = heads_per_core
    hd = hpc * d_head                      # head dims owned per core (=128)
    n_dmc = d_model // P                   # d_model chunks (8)
    n_qt = qlen // P                       # q tiles (16)
    n_tt = (bsz * qlen) // P               # token tiles over all batches (64)
    tok_own = qlen // n_cores              # tokens owned post-RS per batch (256)
    rows_own = bsz * tok_own               # FFN rows per core (1024)
    n_rt = rows_own // P                   # FFN row tiles (8)
    n_dit = d_inner // P                   # d_inner tiles (32)
    W = qlen + P                           # pre-shift row width
    assert hd == 128

    nc = bacc.Bacc("TRN2", num_devices=n_cores, target_bir_lowering=False,
                   detect_race_conditions=False)

    # ---- external inputs (per-core contents prepared by the host) ----
    wT = nc.dram_tensor("wT", [P, (bsz * qlen) * n_dmc], BF16, kind="ExternalInput")
    w_own = nc.dram_tensor("w_own", [rows_own, d_model], F32, kind="ExternalInput")
    rT = nc.dram_tensor("rT", [P, qlen * n_dmc], BF16, kind="ExternalInput")
    wq_t = nc.dram_tensor("wq_t", [P, n_dmc * hd], BF16, kind="ExternalInput")
    wk_t = nc.dram_tensor("wk_t", [P, n_dmc * hd], BF16, kind="ExternalInput")
    wv_t = nc.dram_tensor("wv_t", [P, n_dmc * hd], BF16, kind="ExternalInput")
    wr_t = nc.dram_tensor("wr_t", [P, n_dmc * hd], BF16, kind="ExternalInput")
    bqrw = nc.dram_tensor("bqrw", [hd], F32, kind="ExternalInput")
    bqrr = nc.dram_tensor("bqrr", [hd], F32, kind="ExternalInput")
    bk = nc.dram_tensor("bk", [hd], F32, kind="ExternalInput")
    bv = nc.dram_tensor("bv", [hd], F32, kind="ExternalInput")
    br = nc.dram_tensor("br", [hd], F32, kind="ExternalInput")
    wo_hd = nc.dram_tensor("wo_hd", [hd, d_model], BF16, kind="ExternalInput")
    bo = nc.dram_tensor("bo", [d_model], BF16, kind="ExternalInput")
    ln1_g = nc.dram_tensor("ln1_g", [d_model], BF16, kind="ExternalInput")
    ln1_b = nc.dram_tensor("ln1_b", [d_model], BF16, kind="ExternalInput")
    ln2_g = nc.dram_tensor("ln2_g", [d_model], BF16, kind="ExternalInput")
    ln2_b = nc.dram_tensor("ln2_b", [d_model], BF16, kind="ExternalInput")
    w1_kd = nc.dram_tensor("w1_kd", [P, n_dit * n_dmc * P], BF16, kind="ExternalInput")
    b1_t = nc.dram_tensor("b1_t", [P, n_dit], F32, kind="ExternalInput")
    w2_kd = nc.dram_tensor("w2_kd", [d_inner, d_model], BF16, kind="ExternalInput")
    b2 = nc.dram_tensor("b2", [d_model], BF16, kind="ExternalInput")

    out_own = nc.dram_tensor("out_own", [rows_own, d_model], F32,
                             kind="ExternalOutput")

    cc_in = [nc.dram_tensor(f"cc_in{b}", [qlen, d_model], BF16)
             for b in range(bsz)]
    cc_out = [nc.dram_tensor(f"cc_out{b}", [tok_own, d_model], BF16)
              for b in range(bsz)]
    rgroups = [list(range(n_cores))]

    scale = 1.0 / (d_head ** 0.5)

    from contextlib import ExitStack
    with tile.TileContext(nc) as tc, ExitStack() as _res_ctx:
        res = _res_ctx.enter_context(tc.tile_pool(name="res0", bufs=1))
        ident_bf = res.tile([P, P], BF16)
        make_identity(nc, ident_bf[:])
        ident_f32 = res.tile([P, P], F32)
        make_identity(nc, ident_f32[:])
        bqrw_sb = res.tile([P, 1], F32)
        bqrr_sb = res.tile([P, 1], F32)
        bk_sb = res.tile([P, 1], F32)
        bv_sb = res.tile([P, 1], F32)
        br_sb = res.tile([P, 1], F32)
        for t, d in ((bqrw_sb, bqrw), (bqrr_sb, bqrr), (bk_sb, bk),
                     (bv_sb, bv), (br_sb, br)):
            nc.sync.dma_start(t[:], d[:].unsqueeze(1))
        eps_sb = res.tile([P, 1], F32)
        nc.vector.memset(eps_sb[:], 1e-5)

        # broadcast [P, d_model] copies of per-feature vectors (bf16)
        bo_bc = res.tile([P, d_model], BF16)
        g1_bc = res.tile([P, d_model], BF16)
        b1ln_bc = res.tile([P, d_model], BF16)
        g2_bc = res.tile([P, d_model], BF16)
        b2ln_bc = res.tile([P, d_model], BF16)
        b2_bc = res.tile([P, d_model], BF16)
        for t, d in ((bo_bc, bo), (g1_bc, ln1_g), (b1ln_bc, ln1_b),
                     (g2_bc, ln2_g), (b2ln_bc, ln2_b), (b2_bc, b2)):
            nc.sync.dma_start(t[:], bass.AP(tensor=d[:].tensor, offset=0,
                                            ap=[[0, P], [1, d_model]]))

        # LN1 outputs (residual + transposed normalized bf16 for FFN)
        x1_sb = res.tile([P, n_rt, d_model], BF16)
        x1T_sb = res.tile([P, n_dmc, rows_own], BF16)

        n_ln = max(1, d_model // 512)

        def layer_norm_xn(x_ap, out_xn, pool):
            """out_xn = (x - mu) * rsqrt(var + eps)   (no affine)."""
            stats = pool.tile([P, n_ln, 6], F32, tag="lnstats")
            lw = d_model // n_ln
            for s in range(n_ln):
                nc.vector.bn_stats(stats[:, s, :], x_ap[:, s * lw:(s + 1) * lw])
            mv = pool.tile([P, 2], F32, tag="lnmv")
            nc.vector.bn_aggr(mv[:], stats[:])
            stdv = pool.tile([P, 1], F32, tag="stdv")
            nc.scalar.activation(stdv[:], mv[:, 1:2], AF.Sqrt, bias=eps_sb[:])
            rstd = pool.tile([P, 1], F32, tag="rstd")
            nc.vector.reciprocal(rstd[:], stdv[:])
            nmr = pool.tile([P, 1], F32, tag="nmr")
            nc.vector.tensor_tensor(out=nmr[:], in0=mv[:, 0:1], in1=rstd[:],
                                    op=ALU.mult)
            nc.vector.tensor_scalar(out=nmr[:], in0=nmr[:], scalar1=-1.0,
                                    scalar2=None, op0=ALU.mult)
            nc.scalar.activation(out_xn, x_ap, AF.Identity, bias=nmr[:],
                                 scale=rstd[:])

        for _rep in range(repeat):
            _pcl_ctx = ExitStack()
            pcl = _pcl_ctx.enter_context(tc.tile_pool(name="pc_ln", bufs=1))
            _ab_ctx = ExitStack()
            resAB = _ab_ctx.enter_context(tc.tile_pool(name="resAB", bufs=1))
            # activation streams, live through phases A+B only
            qrw_sb = resAB.tile([P, bsz * qlen], BF16)  # q+bq+r_w_bias [hd,(b,t)]
            qrr_sb = resAB.tile([P, bsz * qlen], BF16)  # q+bq+r_r_bias
            kT_sb = resAB.tile([P, bsz * qlen], BF16)
            rkT_sb = resAB.tile([P, qlen], BF16)
            v_sb = resAB.tile([P, n_tt, hd], BF16)      # [tok%128,(b,t)//128,hd]
            wo_sb = resAB.tile([P, d_model], BF16)
            nc.sync.dma_start(wo_sb[:], wo_hd[:])
            # one PSUM pool shared by phases A and B: no pool boundary, so
            # batch-0 attention can interleave with the tail of phase A
            _pbp_ctx = ExitStack()
            pbp = _pbp_ctx.enter_context(
                tc.tile_pool(name="pb_ps", bufs=1, space="PSUM"))
            # ---------------- phase A: projections ----------------
            with tc.tile_pool(name="pa_sb", bufs=2) as pa, \
                 tc.tile_pool(name="pa_w", bufs=2) as paw, \
                 tc.tile_pool(name="pa_wts", bufs=1) as pawt:
                wq_sb = pawt.tile([P, n_dmc, hd], BF16)
                wk_sb = pawt.tile([P, n_dmc, hd], BF16)
                wv_sb = pawt.tile([P, n_dmc, hd], BF16)
                wr_sb = pawt.tile([P, n_dmc, hd], BF16)
                for t, d in ((wq_sb, wq_t), (wk_sb, wk_t), (wv_sb, wv_t),
                             (wr_sb, wr_t)):
                    nc.sync.dma_start(t[:], d[:])
                # rk projection (needed first by stage1)
                for cs in range(0, qlen, 512):
                    cw = min(512, qlen - cs)
                    rT_c = paw.tile([P, n_dmc, 512], BF16, tag="rTc")
                    nc.sync.dma_start(
                        rT_c[:, :, :cw],
                        rT[:, (cs // 512) * n_dmc * 512:
                           (cs // 512) * n_dmc * 512 + n_dmc * cw])
                    ps = pbp.tile([P, 512], F32, tag="sc", bufs=4)
                    for d in range(n_dmc):
                        nc.tensor.matmul(ps[:, :cw], wr_sb[:, d, :],
                                         rT_c[:, d, :cw],
                                         start=(d == 0), stop=(d == n_dmc - 1))
                    nc.scalar.activation(rkT_sb[:, cs:cs + cw], ps[:, :cw],
                                         AF.Identity, bias=br_sb[:])

                n_ck = (bsz * qlen) // 512
                for c in range(n_ck):
                    cs = c * 512
                    wT_c = paw.tile([P, n_dmc, 512], BF16, tag="wTc", bufs=3)
                    eng = nc.sync if c % 2 == 0 else nc.scalar
                    eng.dma_start(
                        wT_c[:], wT[:, c * n_dmc * 512:(c + 1) * n_dmc * 512])
                    # q -> qrw (ACT) + qrr (DVE)
                    ps = pbp.tile([P, 512], F32, tag="sc", bufs=4)
                    for d in range(n_dmc):
                        nc.tensor.matmul(ps[:], wq_sb[:, d, :], wT_c[:, d, :],
                                         start=(d == 0), stop=(d == n_dmc - 1))
                    nc.scalar.activation(qrw_sb[:, cs:cs + 512], ps[:],
                                         AF.Identity, bias=bqrw_sb[:])
                    nc.vector.tensor_scalar(out=qrr_sb[:, cs:cs + 512], in0=ps[:],
                                            scalar1=bqrr_sb[:], scalar2=None,
                                            op0=ALU.add)
                    # k -> kT (DVE)
                    ps = pbp.tile([P, 512], F32, tag="sc", bufs=4)
                    for d in range(n_dmc):
                        nc.tensor.matmul(ps[:], wk_sb[:, d, :], wT_c[:, d, :],
                                         start=(d == 0), stop=(d == n_dmc - 1))
                    nc.vector.tensor_scalar(out=kT_sb[:, cs:cs + 512], in0=ps[:],
                                            scalar1=bk_sb[:], scalar2=None,
                                            op0=ALU.add)
                    # v -> vT chunk (ACT) -> one batched DMA transpose
                    ps = pbp.tile([P, 512], F32, tag="sc", bufs=4)
                    for d in range(n_dmc):
                        nc.tensor.matmul(ps[:], wv_sb[:, d, :], wT_c[:, d, :],
                                         start=(d == 0), stop=(d == n_dmc - 1))
                    vT_c = pa.tile([P, 512], BF16, tag="vTc")
                    nc.scalar.activation(vT_c[:], ps[:], AF.Identity, bias=bv_sb[:])
                    # PE transpose (DMA transpose would serialize against the
                    # runtime's comm-init collective)
                    tpv = pbp.tile([P, 512], BF16, tag="tps", bufs=2)
                    for k in range(4):
                        nc.tensor.transpose(tpv[:, k * P:(k + 1) * P],
                                            vT_c[:, k * P:(k + 1) * P],
                                            ident_bf[:])
                    nc.vector.tensor_copy(v_sb[:, c * 4:(c + 1) * 4, :], tpv[:])

            # ---------------- phase B: attention (+ staggered LN1) ----------
            _b_ctx = ExitStack()
            pb = _b_ctx.enter_context(tc.tile_pool(name="pb_sb", bufs=2))
            pbs = _b_ctx.enter_context(tc.tile_pool(name="pb_sm", bufs=2))

            def skew_src(pre2, L):
                return bass.AP(tensor=pre2.tensor,
                               offset=pre2[:].offset + (P - 1),
                               ap=[[hpc * W - 1, P], [W, hpc], [1, L]])

            def skew_src_h(pre2, L, hl):
                return bass.AP(tensor=pre2.tensor,
                               offset=pre2[:].offset + (P - 1) + hl * W,
                               ap=[[hpc * W - 1, P], [1, L]])

            def stage1(b, I):
                """BD pre-shift scores for both heads -> pre2 (+NEG pad)."""
                boff = b * qlen
                L = P * (I + 1)
                pre2 = pbs.tile([P, hpc, W], BF16, tag="pre", bufs=2, name="pre2")
                if I < 2:
                    # first use of this rotating buffer in the batch: NEG-fill
                    # everything beyond the BD region once; later iterations
                    # (same buffer, larger L) only ever overwrite [0, L) so
                    # the pad at [L, L+P) stays NEG without a per-I memset
                    nc.vector.memset(pre2[:, :, L:], NEG_BIG)
                n_ch = (L + 511) // 512
                for hl in range(hpc):
                    hsl = slice(hl * d_head, (hl + 1) * d_head)
                    qrr = qrr_sb[hsl, boff + I * P:boff + (I + 1) * P]
                    for c in range(n_ch):
                        cw = min(512, L - c * 512)
                        bdp = pbp.tile([P, 512], F32, tag="sc", bufs=4, name="bdp")
                        nc.tensor.matmul(bdp[:, :cw], qrr,
                                         rkT_sb[hsl, qlen - L + c * 512:
                                                qlen - L + c * 512 + cw],
                                         start=True, stop=True,
                                         tile_position=((hl * d_head, 0)
                                                        if row_tile else None))
                        nc.vector.tensor_copy(pre2[:, hl, c * 512:c * 512 + cw],
                                              bdp[:, :cw])
                return pre2

            def stage2a(b, I, pre2):
                boff = b * qlen
                L = P * (I + 1)
                qs = boff + I * P
                n_ch = (L + 511) // 512
                s2 = pbs.tile([P, hpc, qlen], BF16, tag="s2", bufs=2, name="s2")
                if not skew_accum:
                    bdsk = pbs.tile([P, hpc, qlen], BF16, tag="bdsk", bufs=2,
                                    name="bdsk")
                    nc.sync.dma_start(bdsk[:, :, :L], skew_src(pre2, L))
                for hl in range(hpc):
                    hsl = slice(hl * d_head, (hl + 1) * d_head)
                    qrw = qrw_sb[hsl, qs:qs + P]
                    for c in range(n_ch):
                        cw = min(512, L - c * 512)
                        acp = pbp.tile([P, 512], F32, tag="sc", bufs=4, name="acp")
                        nc.tensor.matmul(acp[:, :cw], qrw,
                                         kT_sb[hsl, boff + c * 512:boff + c * 512 + cw],
                                         start=True, stop=True,
                                         tile_position=((hl * d_head, 0)
                                                        if row_tile else None))
                        if skew_accum:
                            if hl == 0:
                                nc.scalar.copy(s2[:, hl, c * 512:c * 512 + cw],
                                               acp[:, :cw])
                            else:
                                nc.vector.tensor_copy(
                                    s2[:, hl, c * 512:c * 512 + cw], acp[:, :cw])
                        else:
                            nc.vector.tensor_tensor(
                                out=s2[:, hl, c * 512:c * 512 + cw],
                                in0=acp[:, :cw],
                                in1=bdsk[:, hl, c * 512:c * 512 + cw],
                                op=ALU.add)
                prob = pbs.tile([P, 2 * qlen], BF16, tag="prob", bufs=2,
                                name="prob")
                rinvs = []
                for hl in range(hpc):
                    if skew_accum:
                        # s2 += rel-shifted BD (carries NEG mask via the pad);
                        # per-head so exp(h0) overlaps the h1 skew
                        nc.gpsimd.dma_start(s2[:, hl, :L],
                                            skew_src_h(pre2, L, hl),
                                            accum_op=ALU.add)
                    rsum = pb.tile([P, 1], F32, tag=f"rsum{hl}", name="rsum")
                    nc.scalar.activation(prob[:, hl * L:(hl + 1) * L],
                                         s2[:, hl, :L], AF.Exp,
                                         scale=scale, accum_out=rsum[:])
                    rinv = pb.tile([P, 1], F32, tag=f"rinv{hl}", name="rinv")
                    nc.vector.reciprocal(rinv[:], rsum[:])
                    rinvs.append(rinv)
                return (I, L, prob, rinvs)

            def stage2b(b, ctx, avT_b):
                """prob transpose + PV + av for row-tile I (one tile behind
                stage2a, so the PE queue never idles on the exp chain)."""
                I, L, prob, rinvs = ctx
                # transpose prob via the PE (DMA transpose would serialize
                # against in-flight collectives), 4 tiles per PSUM evict
                pts = pbs.tile([P, 2 * n_qt, P], BF16, tag="pts", bufs=2,
                               name="pts")
                for hl in range(hpc):
                    for g in range(0, I + 1, 4):
                        gn = min(4, I + 1 - g)
                        tps = pbp.tile([P, 512], BF16, tag="tps", bufs=2,
                                       name="tps")
                        for k in range(gn):
                            nc.tensor.transpose(
                                tps[:, k * P:(k + 1) * P],
                                prob[:, hl * L + (g + k) * P:
                                     hl * L + (g + k + 1) * P],
                                ident_bf[:])
                        dst = pts[:, hl * (I + 1) + g:hl * (I + 1) + g + gn, :]
                        if (g // 4 + hl) % 3 != 0:
                            nc.scalar.copy(dst, tps[:, :gn * P])
                        else:
                            nc.vector.tensor_copy(dst, tps[:, :gn * P])
                pv = pbp.tile([P, hd], F32, tag="pv", bufs=1, name="pv")
                av = pb.tile([P, hd], BF16, tag="av", name="av")
                for hl in range(hpc):
                    hsl = slice(hl * d_head, (hl + 1) * d_head)
                    for J in range(I + 1):
                        nc.tensor.matmul(pv[:, hsl], pts[:, hl * (I + 1) + J, :],
                                         v_sb[:, b * n_qt + J, hsl],
                                         start=(J == 0), stop=(J == I),
                                         skip_group_check=True)
                    nc.vector.tensor_scalar(out=av[:, hsl], in0=pv[:, hsl],
                                            scalar1=rinvs[hl][:], scalar2=None,
                                            op0=ALU.mult)
                tp = pbp.tile([P, 512], BF16, tag="tps", bufs=2, name="avtp")
                nc.tensor.transpose(tp[:, :P], av[:], ident_bf[:])
                nc.scalar.copy(avT_b[:, I * P:(I + 1) * P], tp[:, :P])

            def emit_ln1(b, ppool):
                """residual + LN1 for batch b's owned rows (cc_out[b] ready)."""
                for rt2 in range(tok_own // P):
                    rt = b * (tok_own // P) + rt2
                    rs_bf = pcl.tile([P, d_model], BF16, tag="rsbf")
                    nc.sync.dma_start(rs_bf[:],
                                      cc_out[b][rt2 * P:(rt2 + 1) * P, :])
                    wres = pcl.tile([P, d_model], F32, tag="wres")
                    nc.sync.dma_start(wres[:], w_own[rt * P:(rt + 1) * P, :])
                    nc.vector.tensor_tensor(out=wres[:], in0=wres[:],
                                            in1=rs_bf[:], op=ALU.add)
                    nc.vector.tensor_tensor(out=wres[:], in0=wres[:],
                                            in1=bo_bc[:], op=ALU.add)
                    xn = pcl.tile([P, d_model], F32, tag="xn")
                    layer_norm_xn(wres[:], xn[:], pcl)
                    # transposed normalized copy for the FFN (g folded into W1)
                    for dt in range(n_dmc):
                        xt = ppool.tile([P, P], F32, tag="tpc", bufs=1, name="xt")
                        nc.tensor.transpose(xt[:], xn[:, dt * P:(dt + 1) * P],
                                            ident_f32[:])
                        nc.vector.tensor_copy(
                            x1T_sb[:, dt, rt * P:(rt + 1) * P], xt[:])
                    # full LN1 output for the residual path
                    nc.gpsimd.tensor_tensor(out=xn[:], in0=xn[:], in1=g1_bc[:],
                                            op=ALU.mult)
                    nc.gpsimd.tensor_tensor(out=x1_sb[:, rt, :], in0=xn[:],
                                            in1=b1ln_bc[:], op=ALU.add)

            bdsk_hold = [None]
            for b in range(bsz):
                boff = b * qlen
                avT_b = pb.tile([P, qlen], BF16, tag="avT", bufs=2,
                                name=f"avT{b}")
                carry = stage1(b, 0)
                pend = None
                for I in range(n_qt):
                    nxt = stage1(b, I + 1) if I + 1 < n_qt else None
                    cur = stage2a(b, I, carry)
                    if pend is not None:
                        stage2b(b, pend, avT_b)
                    carry = nxt
                    pend = cur
                stage2b(b, pend, avT_b)
                # o-projection partial for this batch + ReduceScatter
                for T in range(n_qt):
                    wo_bf = pb.tile([P, d_model], BF16, tag="wobf", bufs=2)
                    for gs in range(0, d_model, 512):
                        gw = min(512, d_model - gs)
                        wop = pbp.tile([P, 512], F32, tag="sc", bufs=4)
                        nc.tensor.matmul(wop[:, :gw],
                                         avT_b[:, T * P:(T + 1) * P],
                                         wo_sb[:, gs:gs + gw],
                                         start=True, stop=True)
                        if (T + gs // 512) % 2 == 0:
                            nc.scalar.copy(wo_bf[:, gs:gs + gw], wop[:, :gw])
                        else:
                            nc.vector.tensor_copy(wo_bf[:, gs:gs + gw],
                                                  wop[:, :gw])
                    nc.sync.dma_start(cc_in[b][T * P:(T + 1) * P, :], wo_bf[:])
                nc.gpsimd.collective_compute(
                    "ReduceScatter", ALU.add, replica_groups=rgroups,
                    ins=[cc_in[b][:]], outs=[cc_out[b][:]])
            # phase C: residual + LN1.  The virtual-time floor keeps the
            # scheduler from backfilling these RS-dependent ops into idle
            # slots mid-attention (their sem waits would head-of-line-block
            # the vector/sync queues while a ReduceScatter is in flight).
            with tc.tile_wait_until(50):
                for b in range(2):
                    emit_ln1(b, pbp)
            _b_ctx.close()
            _pbp_ctx.close()
            _ab_ctx.close()

            # ---------------- phase D: FFN + LN2 ----------------
            _d_ctx = ExitStack()
            resD = _d_ctx.enter_context(tc.tile_pool(name="resD", bufs=1))
            hT_sb = resD.tile([P, n_dit, rows_own], BF16)
            b1_sb = resD.tile([P, n_dit], F32)
            nc.sync.dma_start(b1_sb[:], b1_t[:])
            # FFN1 in row-halves: the first half only needs LN1 of batches
            # 0-1, so it streams while RS(3)/LN1(2,3) are still finishing
            with tc.tile_pool(name="pd_w", bufs=3) as pdw, \
                 tc.tile_pool(name="pd_ps", bufs=2, space="PSUM") as pdp:
                for half in range(2):
                    ts = half * 512
                    for dt in range(n_dit):
                        w1c = pdw.tile([P, n_dmc, P], BF16, tag="w1c")
                        nc.sync.dma_start(
                            w1c[:], w1_kd[:, dt * n_dmc * P:(dt + 1) * n_dmc * P])
                        ps = pdp.tile([P, 512], F32, tag="ffn1")
                        for d in range(n_dmc):
                            nc.tensor.matmul(
                                ps[:], w1c[:, d, :],
                                x1T_sb[:, d, ts:ts + 512],
                                start=(d == 0), stop=(d == n_dmc - 1))
                        nc.scalar.activation(
                            hT_sb[:, dt, ts:ts + 512], ps[:],
                            AF.Relu, bias=b1_sb[:, dt:dt + 1])
                    if half == 0:
                        with tc.tile_wait_until(50):
                            for b in range(2, bsz):
                                emit_ln1(b, pdp)

            with tc.tile_pool(name="pe_w", bufs=3) as pew, \
                 tc.tile_pool(name="pe_sb", bufs=3) as pes, \
                 tc.tile_pool(name="pe_ps", bufs=4, space="PSUM") as pep:
                for half in range(2):
                    rts = range(half * (n_rt // 2), (half + 1) * (n_rt // 2))
                    psy = {rt: pep.tile([P, d_model], F32, tag=f"ffn2_{rt % 4}",
                                        name=f"psy{rt}", bufs=1)
                           for rt in rts}
                    for dt in range(n_dit):
                        w2c = pew.tile([P, d_model], BF16, tag="w2c")
                        nc.sync.dma_start(w2c[:], w2_kd[dt * P:(dt + 1) * P, :])
                        for rt in rts:
                            for gs in range(0, d_model, 512):
                                gw = min(512, d_model - gs)
                                nc.tensor.matmul(
                                    psy[rt][:, gs:gs + gw],
                                    hT_sb[:, dt, rt * P:(rt + 1) * P],
                                    w2c[:, gs:gs + gw],
                                    start=(dt == 0), stop=(dt == n_dit - 1))
                    for rt in rts:
                        y_sb = pes.tile([P, d_model], F32, tag="ysb")
                        nc.vector.tensor_tensor(out=y_sb[:], in0=psy[rt][:],
                                                in1=x1_sb[:, rt, :], op=ALU.add)
                        nc.vector.tensor_tensor(out=y_sb[:], in0=y_sb[:],
                                                in1=b2_bc[:], op=ALU.add)
                        o_sb = pes.tile([P, d_model], F32, tag="osb")
                        layer_norm_xn(y_sb[:], o_sb[:], pes)
                        nc.gpsimd.tensor_tensor(out=o_sb[:], in0=o_sb[:],
                                                in1=g2_bc[:], op=ALU.mult)
                        nc.gpsimd.tensor_tensor(out=o_sb[:], in0=o_sb[:],
                                                in1=b2ln_bc[:], op=ALU.add)
                        nc.sync.dma_start(out_own[rt * P:(rt + 1) * P, :], o_sb[:])
            _d_ctx.close()
            _pcl_ctx.close()
    nc.finalize()
    return nc


# ----------------------------------------------------------------------------
# host-side wrapper
# ----------------------------------------------------------------------------
QLEN, BSZ, D_MODEL, N_HEAD, D_HEAD, D_INNER = 2048, 4, 1024, 16, 64, 4096
N_CORES = 8
HPC = N_HEAD // N_CORES

_prog_cache = {}


def _get_program():
    key = "full"
    if key not in _prog_cache:
        _prog_cache[key] = build_program(QLEN, BSZ, D_MODEL, D_HEAD, HPC,
                                         D_INNER, N_CORES)
    return _prog_cache[key]


def make_in_maps(w, r, Wq, bq, Wk, bk, Wv, bv, Wr, br, Wo, bo,
                 r_w_bias, r_r_bias, ln1_g, ln1_b, W1, b1, W2, b2,
                 qlen=QLEN, bsz=BSZ, d_model=D_MODEL, d_head=D_HEAD,
                 hpc=HPC, d_inner=D_INNER, n_cores=N_CORES,
                 ln2_g=None, ln2_b=None):
    f32 = np.float32
    import ml_dtypes
    bf16 = ml_dtypes.bfloat16
    n_dmc = d_model // 128
    wTf = w.transpose(2, 1, 0).reshape(d_model, bsz * qlen)
    wT = np.ascontiguousarray(
        wTf.reshape(n_dmc, 128, (bsz * qlen) // 512, 512)
        .transpose(1, 2, 0, 3).reshape(128, -1)).astype(bf16)
    rT = np.ascontiguousarray(
        np.asarray(r).T.reshape(n_dmc, 128, qlen // 512, 512)
        .transpose(1, 2, 0, 3).reshape(128, -1)).astype(bf16)
    tok_own = qlen // n_cores
    n_dit = d_inner // 128
    # fold LN1 affine into W1/b1 (FFN path consumes the un-affined xn)
    W1f = np.asarray(W1, f32) * np.asarray(ln1_g, f32)[None, :]
    b1f = np.asarray(b1, f32) + np.asarray(W1, f32) @ np.asarray(ln1_b, f32)
    w1_kd = np.ascontiguousarray(
        W1f.T.reshape(n_dmc, 128, n_dit, 128)
        .transpose(1, 2, 0, 3).reshape(128, -1)).astype(bf16)
    w2_kd = np.ascontiguousarray(W2.T).astype(bf16)
    b1_t = np.ascontiguousarray(b1f.reshape(n_dit, 128).T).astype(f32)
    common = dict(
        wT=wT, rT=rT, bo=bo.astype(bf16),
        ln1_g=ln1_g.astype(bf16), ln1_b=ln1_b.astype(bf16),
        ln2_g=ln2_g.astype(bf16), ln2_b=ln2_b.astype(bf16),
        w1_kd=w1_kd, b1_t=b1_t, w2_kd=w2_kd, b2=b2.astype(bf16),
    )
    in_maps = []
    for c in range(n_cores):
        hs = slice(c * hpc * d_head, (c + 1) * hpc * d_head)
        head_sl = slice(c * hpc, (c + 1) * hpc)
        tok_sl = slice(c * tok_own, (c + 1) * tok_own)
        w_own = np.ascontiguousarray(
            w[tok_sl].transpose(1, 0, 2).reshape(bsz * tok_own, d_model)
        ).astype(f32)
        rwb = np.asarray(r_w_bias, f32)[head_sl].reshape(-1)
        rrb = np.asarray(r_r_bias, f32)[head_sl].reshape(-1)
        m = dict(common)
        m.update(
            w_own=w_own,
            wq_t=np.ascontiguousarray(np.asarray(Wq)[hs].T.reshape(n_dmc, 128, 128).transpose(1, 0, 2).reshape(128, -1)).astype(bf16),
            wk_t=np.ascontiguousarray(np.asarray(Wk)[hs].T.reshape(n_dmc, 128, 128).transpose(1, 0, 2).reshape(128, -1)).astype(bf16),
            wv_t=np.ascontiguousarray(np.asarray(Wv)[hs].T.reshape(n_dmc, 128, 128).transpose(1, 0, 2).reshape(128, -1)).astype(bf16),
            wr_t=np.ascontiguousarray(np.asarray(Wr)[hs].T.reshape(n_dmc, 128, 128).transpose(1, 0, 2).reshape(128, -1)).astype(bf16),
            bqrw=(bq[hs].astype(f32) + rwb).astype(f32),
            bqrr=(bq[hs].astype(f32) + rrb).astype(f32),
            bk=bk[hs].astype(f32),
            bv=bv[hs].astype(f32), br=br[hs].astype(f32),
            wo_hd=np.ascontiguousarray(Wo[:, hs].T).astype(bf16),
        )
        in_maps.append(m)
    return in_maps


def assemble_output(results, qlen=QLEN, bsz=BSZ, d_model=D_MODEL,
                    n_cores=N_CORES):
    tok_own = qlen // n_cores
    out = np.empty((qlen, bsz, d_model), np.float32)
    for c in range(n_cores):
        slab = results[c]["out_own"].reshape(bsz, tok_own, d_model)
        out[c * tok_own:(c + 1) * tok_own] = slab.transpose(1, 0, 2)
    return out


def kernel(w, r, attn_mask, Wq, bq, Wk, bk, Wv, bv, Wr, br, Wo, bo,
           r_w_bias, r_r_bias, ln1_g, ln1_b, W1, b1, W2, b2, ln2_g, ln2_b):
    w = np.asarray(w); r = np.asarray(r)
    attn_mask = np.asarray(attn_mask)
    expect = np.triu(np.ones((QLEN, QLEN), dtype=bool), k=1)
    assert np.array_equal(attn_mask, expect), \
        "kernel specializes the causal (triu) attention mask"
    from concourse.bass_utils import run_bass_kernel_spmd
    nc = _get_program()
    in_maps = make_in_maps(np.asarray(w), np.asarray(r),
                           np.asarray(Wq), np.asarray(bq), np.asarray(Wk),
                           np.asarray(bk), np.asarray(Wv), np.asarray(bv),
                           np.asarray(Wr), np.asarray(br), np.asarray(Wo),
                           np.asarray(bo), np.asarray(r_w_bias),
                           np.asarray(r_r_bias), np.asarray(ln1_g),
                           np.asarray(ln1_b), np.asarray(W1), np.asarray(b1),
                           np.asarray(W2), np.asarray(b2),
                           ln2_g=np.asarray(ln2_g), ln2_b=np.asarray(ln2_b))
    res = run_bass_kernel_spmd(nc, in_maps, list(range(N_CORES)))
    return assemble_output(res.results)


# revision 46
# speedup vs baseline: 1.0088x; 1.0088x over previous
"""Trainium2 Bass kernel for nn_MemTransformerLM (Transformer-XL layer).

Sharding (8 NeuronCores): tensor-parallel over heads -- each core owns 2 of the
16 heads for all 4 batches, computes q/k/v/r projections for its heads, full
causal rel-attention for its (batch, head) pairs, and the column-slice of the
o-projection.  Per-batch partial attention outputs are summed across cores with
a ReduceScatter (bf16) that also hands each core a 256-token slice of every
batch; LayerNorm1 + position-wise FFN + LayerNorm2 then run token-parallel on
those rows with no further communication.  The host reassembles the full
[qlen, bsz, d_model] output from the per-core row slabs.

Key structure (vs the v1 baseline, 1885us -> ~1460us):
 - all activation streams bf16 (wT/rT shipped bf16, halving input DMA)
 - q+r_w_bias / q+r_r_bias / k+bk are folded into the projection-PSUM
   evictions in phase A (replaces 234us of per-batch gpsimd broadcast adds)
 - the rel-shift pad is NEG_BIG-filled once per buffer per batch, so causal
   masking falls out of the skew read -- no affine_select, no per-I memset
 - the skew is a per-head SWDGE accumulate-DMA that adds shifted BD directly
   into the AC score tile (no separate DVE add pass); exp(h0) overlaps the
   h1 skew
 - score matmuls for the core's two heads run concurrently in the PE array
   via row tiling (K=64 each)
 - NO DMA transposes anywhere: prob/av/v transposes run on the PE (the
   runtime serializes every DMA transpose against in-flight collectives --
   the comm-init collective and each ReduceScatter would stall them 40-70us)
 - ReduceScatter overlaps the next batch's attention; LayerNorm1 is floored
   past the attention region with tile_wait_until so the scheduler cannot
   backfill its RS-dependent ops into attention-era queue slots (their sem
   waits head-of-line-block the vector/sync queues)
"""

import sys

for _p in ("/opt/trn_rl_repo", "/root/.axon_site/_ro/trn_rl_repo"):
    if _p not in sys.path:
        sys.path.insert(0, _p)

import numpy as np

import concourse.bass as bass
import concourse.mybir as mybir
import concourse.tile as tile
from concourse import bacc
from concourse.masks import make_identity

F32 = mybir.dt.float32
BF16 = mybir.dt.bfloat16
AF = mybir.ActivationFunctionType
ALU = mybir.AluOpType

NEG_BIG = -30000.0


def build_program(qlen=2048, bsz=4, d_model=1024, d_head=64, heads_per_core=2,
                  d_inner=4096, n_cores=8, repeat=1, row_tile=True,
                  skew_accum=True):
    P = 128
    hpc = heads_per_core
    hd = hpc * d_head                      # head dims owned per core (=128)
    n_dmc = d_model // P                   # d_model chunks (8)
    n_qt = qlen // P                       # q tiles (16)
    n_tt = (bsz * qlen) // P               # token tiles over all batches (64)
    tok_own = qlen // n_cores              # tokens owned post-RS per batch (256)
    rows_own = bsz * tok_own               # FFN rows per core (1024)
    n_rt = rows_own // P                   # FFN row tiles (8)
    n_dit = d_inner // P                   # d_inner tiles (32)
    W = qlen + P                           # pre-shift row width
    assert hd == 128

    nc = bacc.Bacc("TRN2", num_devices=n_cores, target_bir_lowering=False,
                   detect_race_conditions=False)

    # ---- external inputs (per-core contents prepared by the host) ----
    wT = nc.dram_tensor("wT", [P, (bsz * qlen) * n_dmc], BF16, kind="ExternalInput")
    w_own = nc.dram_tensor("w_own", [rows_own, d_model], F32, kind="ExternalInput")
    rT = nc.dram_tensor("rT", [P, qlen * n_dmc], BF16, kind="ExternalInput")
    wq_t = nc.dram_tensor("wq_t", [P, n_dmc * hd], BF16, kind="ExternalInput")
    wk_t = nc.dram_tensor("wk_t", [P, n_dmc * hd], BF16, kind="ExternalInput")
    wv_t = nc.dram_tensor("wv_t", [P, n_dmc * hd], BF16, kind="ExternalInput")
    wr_t = nc.dram_tensor("wr_t", [P, n_dmc * hd], BF16, kind="ExternalInput")
    bqrw = nc.dram_tensor("bqrw", [hd], F32, kind="ExternalInput")
    bqrr = nc.dram_tensor("bqrr", [hd], F32, kind="ExternalInput")
    bk = nc.dram_tensor("bk", [hd], F32, kind="ExternalInput")
    bv = nc.dram_tensor("bv", [hd], F32, kind="ExternalInput")
    br = nc.dram_tensor("br", [hd], F32, kind="ExternalInput")
    wo_hd = nc.dram_tensor("wo_hd", [hd, d_model], BF16, kind="ExternalInput")
    bo = nc.dram_tensor("bo", [d_model], BF16, kind="ExternalInput")
    ln1_g = nc.dram_tensor("ln1_g", [d_model], BF16, kind="ExternalInput")
    ln1_b = nc.dram_tensor("ln1_b", [d_model], BF16, kind="ExternalInput")
    ln2_g = nc.dram_tensor("ln2_g", [d_model], BF16, kind="ExternalInput")
    ln2_b = nc.dram_tensor("ln2_b", [d_model], BF16, kind="ExternalInput")
    w1_kd = nc.dram_tensor("w1_kd", [P, n_dit * n_dmc * P], BF16, kind="ExternalInput")
    b1_t = nc.dram_tensor("b1_t", [P, n_dit], F32, kind="ExternalInput")
    w2_kd = nc.dram_tensor("w2_kd", [d_inner, d_model], BF16, kind="ExternalInput")
    b2 = nc.dram_tensor("b2", [d_model], BF16, kind="ExternalInput")

    out_own = nc.dram_tensor("out_own", [rows_own, d_model], F32,
                             kind="ExternalOutput")

    cc_in = [nc.dram_tensor(f"cc_in{b}", [qlen, d_model], BF16)
             for b in range(bsz)]
    cc_out = [nc.dram_tensor(f"cc_out{b}", [tok_own, d_model], BF16)
              for b in range(bsz)]
    rgroups = [list(range(n_cores))]

    scale = 1.0 / (d_head ** 0.5)

    from contextlib import ExitStack
    with tile.TileContext(nc) as tc, ExitStack() as _res_ctx:
        res = _res_ctx.enter_context(tc.tile_pool(name="res0", bufs=1))
        ident_bf = res.tile([P, P], BF16)
        make_identity(nc, ident_bf[:])
        ident_f32 = res.tile([P, P], F32)
        make_identity(nc, ident_f32[:])
        bqrw_sb = res.tile([P, 1], F32)
        bqrr_sb = res.tile([P, 1], F32)
        bk_sb = res.tile([P, 1], F32)
        bv_sb = res.tile([P, 1], F32)
        br_sb = res.tile([P, 1], F32)
        for t, d in ((bqrw_sb, bqrw), (bqrr_sb, bqrr), (bk_sb, bk),
                     (bv_sb, bv), (br_sb, br)):
            nc.sync.dma_start(t[:], d[:].unsqueeze(1))
        eps_sb = res.tile([P, 1], F32)
        nc.vector.memset(eps_sb[:], 1e-5)

        # broadcast [P, d_model] copies of per-feature vectors (bf16)
        bo_bc = res.tile([P, d_model], BF16)
        g1_bc = res.tile([P, d_model], BF16)
        b1ln_bc = res.tile([P, d_model], BF16)
        g2_bc = res.tile([P, d_model], BF16)
        b2ln_bc = res.tile([P, d_model], BF16)
        b2_bc = res.tile([P, d_model], BF16)
        for t, d in ((bo_bc, bo), (g1_bc, ln1_g), (b1ln_bc, ln1_b),
                     (g2_bc, ln2_g), (b2ln_bc, ln2_b), (b2_bc, b2)):
            nc.sync.dma_start(t[:], bass.AP(tensor=d[:].tensor, offset=0,
                                            ap=[[0, P], [1, d_model]]))

        # LN1 outputs (residual + transposed normalized bf16 for FFN)
        x1_sb = res.tile([P, n_rt, d_model], BF16)
        x1T_sb = res.tile([P, n_dmc, rows_own], BF16)

        n_ln = max(1, d_model // 512)

        def layer_norm_xn(x_ap, out_xn, pool):
            """out_xn = (x - mu) * rsqrt(var + eps)   (no affine)."""
            stats = pool.tile([P, n_ln, 6], F32, tag="lnstats")
            lw = d_model // n_ln
            for s in range(n_ln):
                nc.vector.bn_stats(stats[:, s, :], x_ap[:, s * lw:(s + 1) * lw])
            mv = pool.tile([P, 2], F32, tag="lnmv")
            nc.vector.bn_aggr(mv[:], stats[:])
            stdv = pool.tile([P, 1], F32, tag="stdv")
            nc.scalar.activation(stdv[:], mv[:, 1:2], AF.Sqrt, bias=eps_sb[:])
            rstd = pool.tile([P, 1], F32, tag="rstd")
            nc.vector.reciprocal(rstd[:], stdv[:])
            nmr = pool.tile([P, 1], F32, tag="nmr")
            nc.vector.tensor_tensor(out=nmr[:], in0=mv[:, 0:1], in1=rstd[:],
                                    op=ALU.mult)
            nc.vector.tensor_scalar(out=nmr[:], in0=nmr[:], scalar1=-1.0,
                                    scalar2=None, op0=ALU.mult)
            nc.scalar.activation(out_xn, x_ap, AF.Identity, bias=nmr[:],
                                 scale=rstd[:])

        for _rep in range(repeat):
            _pcl_ctx = ExitStack()
            pcl = _pcl_ctx.enter_context(tc.tile_pool(name="pc_ln", bufs=1))
            _ab_ctx = ExitStack()
            resAB = _ab_ctx.enter_context(tc.tile_pool(name="resAB", bufs=1))
            # activation streams, live through phases A+B only
            qrw_sb = resAB.tile([P, bsz * qlen], BF16)  # q+bq+r_w_bias [hd,(b,t)]
            qrr_sb = resAB.tile([P, bsz * qlen], BF16)  # q+bq+r_r_bias
            kT_sb = resAB.tile([P, bsz * qlen], BF16)
            rkT_sb = resAB.tile([P, qlen], BF16)
            v_sb = resAB.tile([P, n_tt, hd], BF16)      # [tok%128,(b,t)//128,hd]
            wo_sb = resAB.tile([P, d_model], BF16)
            nc.sync.dma_start(wo_sb[:], wo_hd[:])
            # one PSUM pool shared by phases A and B: no pool boundary, so
            # batch-0 attention can interleave with the tail of phase A
            _pbp_ctx = ExitStack()
            pbp = _pbp_ctx.enter_context(
                tc.tile_pool(name="pb_ps", bufs=1, space="PSUM"))
            # ---------------- phase A: projections ----------------
            with tc.tile_pool(name="pa_sb", bufs=2) as pa, \
                 tc.tile_pool(name="pa_w", bufs=2) as paw, \
                 tc.tile_pool(name="pa_wts", bufs=1) as pawt:
                wq_sb = pawt.tile([P, n_dmc, hd], BF16)
                wk_sb = pawt.tile([P, n_dmc, hd], BF16)
                wv_sb = pawt.tile([P, n_dmc, hd], BF16)
                wr_sb = pawt.tile([P, n_dmc, hd], BF16)
                for t, d in ((wq_sb, wq_t), (wk_sb, wk_t), (wv_sb, wv_t),
                             (wr_sb, wr_t)):
                    nc.sync.dma_start(t[:], d[:])
                # rk projection, highest chunk first: stage1 of row-tile I
                # reads rkT[qlen-L:qlen], so batch-0 attention gates on the
                # LAST columns -- computing them first starts it earlier
                for ci, cs in enumerate(reversed(range(0, qlen, 512))):
                    cw = min(512, qlen - cs)
                    rT_c = paw.tile([P, n_dmc, 512], BF16, tag="rTc")
                    reng = nc.sync if ci % 2 == 0 else nc.scalar
                    reng.dma_start(
                        rT_c[:, :, :cw],
                        rT[:, (cs // 512) * n_dmc * 512:
                           (cs // 512) * n_dmc * 512 + n_dmc * cw])
                    ps = pbp.tile([P, 512], F32, tag="sc", bufs=4)
                    for d in range(n_dmc):
                        nc.tensor.matmul(ps[:, :cw], wr_sb[:, d, :],
                                         rT_c[:, d, :cw],
                                         start=(d == 0), stop=(d == n_dmc - 1))
                    nc.scalar.activation(rkT_sb[:, cs:cs + cw], ps[:, :cw],
                                         AF.Identity, bias=br_sb[:])

                n_ck = (bsz * qlen) // 512
                for c in range(n_ck):
                    cs = c * 512
                    wT_c = paw.tile([P, n_dmc, 512], BF16, tag="wTc", bufs=3)
                    eng = nc.sync if c % 2 == 0 else nc.scalar
                    eng.dma_start(
                        wT_c[:], wT[:, c * n_dmc * 512:(c + 1) * n_dmc * 512])
                    # q -> qrw (ACT) + qrr (DVE)
                    ps = pbp.tile([P, 512], F32, tag="sc", bufs=4)
                    for d in range(n_dmc):
                        nc.tensor.matmul(ps[:], wq_sb[:, d, :], wT_c[:, d, :],
                                         start=(d == 0), stop=(d == n_dmc - 1))
                    nc.scalar.activation(qrw_sb[:, cs:cs + 512], ps[:],
                                         AF.Identity, bias=bqrw_sb[:])
                    nc.vector.tensor_scalar(out=qrr_sb[:, cs:cs + 512], in0=ps[:],
                                            scalar1=bqrr_sb[:], scalar2=None,
                                            op0=ALU.add)
                    # k -> kT (DVE)
                    ps = pbp.tile([P, 512], F32, tag="sc", bufs=4)
                    for d in range(n_dmc):
                        nc.tensor.matmul(ps[:], wk_sb[:, d, :], wT_c[:, d, :],
                                         start=(d == 0), stop=(d == n_dmc - 1))
                    nc.vector.tensor_scalar(out=kT_sb[:, cs:cs + 512], in0=ps[:],
                                            scalar1=bk_sb[:], scalar2=None,
                                            op0=ALU.add)
                    # v -> vT chunk (ACT) -> one batched DMA transpose
                    ps = pbp.tile([P, 512], F32, tag="sc", bufs=4)
                    for d in range(n_dmc):
                        nc.tensor.matmul(ps[:], wv_sb[:, d, :], wT_c[:, d, :],
                                         start=(d == 0), stop=(d == n_dmc - 1))
                    vT_c = pa.tile([P, 512], BF16, tag="vTc")
                    nc.scalar.activation(vT_c[:], ps[:], AF.Identity, bias=bv_sb[:])
                    # PE transpose (DMA transpose would serialize against the
                    # runtime's comm-init collective)
                    tpv = pbp.tile([P, 512], BF16, tag="tps", bufs=2)
                    for k in range(4):
                        nc.tensor.transpose(tpv[:, k * P:(k + 1) * P],
                                            vT_c[:, k * P:(k + 1) * P],
                                            ident_bf[:])
                    nc.vector.tensor_copy(v_sb[:, c * 4:(c + 1) * 4, :], tpv[:])

            # ---------------- phase B: attention (+ staggered LN1) ----------
            _b_ctx = ExitStack()
            pb = _b_ctx.enter_context(tc.tile_pool(name="pb_sb", bufs=2))
            pbs = _b_ctx.enter_context(tc.tile_pool(name="pb_sm", bufs=2))

            def skew_src(pre2, L):
                return bass.AP(tensor=pre2.tensor,
                               offset=pre2[:].offset + (P - 1),
                               ap=[[hpc * W - 1, P], [W, hpc], [1, L]])

            def skew_src_h(pre2, L, hl):
                return bass.AP(tensor=pre2.tensor,
                               offset=pre2[:].offset + (P - 1) + hl * W,
                               ap=[[hpc * W - 1, P], [1, L]])

            def stage1(b, I):
                """BD pre-shift scores for both heads -> pre2 (+NEG pad)."""
                boff = b * qlen
                L = P * (I + 1)
                pre2 = pbs.tile([P, hpc, W], BF16, tag="pre", bufs=2, name="pre2")
                if I < 2:
                    # first use of this rotating buffer in the batch: NEG-fill
                    # everything beyond the BD region once; later iterations
                    # (same buffer, larger L) only ever overwrite [0, L) so
                    # the pad at [L, L+P) stays NEG without a per-I memset
                    nc.vector.memset(pre2[:, :, L:], NEG_BIG)
                n_ch = (L + 511) // 512
                for hl in range(hpc):
                    hsl = slice(hl * d_head, (hl + 1) * d_head)
                    qrr = qrr_sb[hsl, boff + I * P:boff + (I + 1) * P]
                    for c in range(n_ch):
                        cw = min(512, L - c * 512)
                        bdp = pbp.tile([P, 512], F32, tag="sc", bufs=4, name="bdp")
                        nc.tensor.matmul(bdp[:, :cw], qrr,
                                         rkT_sb[hsl, qlen - L + c * 512:
                                                qlen - L + c * 512 + cw],
                                         start=True, stop=True,
                                         tile_position=((hl * d_head, 0)
                                                        if row_tile else None))
                        nc.vector.tensor_copy(pre2[:, hl, c * 512:c * 512 + cw],
                                              bdp[:, :cw])
                return pre2

            def stage2a(b, I, pre2):
                boff = b * qlen
                L = P * (I + 1)
                qs = boff + I * P
                n_ch = (L + 511) // 512
                s2 = pbs.tile([P, hpc, qlen], BF16, tag="s2", bufs=2, name="s2")
                if not skew_accum:
                    bdsk = pbs.tile([P, hpc, qlen], BF16, tag="bdsk", bufs=2,
                                    name="bdsk")
                    nc.sync.dma_start(bdsk[:, :, :L], skew_src(pre2, L))
                for hl in range(hpc):
                    hsl = slice(hl * d_head, (hl + 1) * d_head)
                    qrw = qrw_sb[hsl, qs:qs + P]
                    for c in range(n_ch):
                        cw = min(512, L - c * 512)
                        acp = pbp.tile([P, 512], F32, tag="sc", bufs=4, name="acp")
                        nc.tensor.matmul(acp[:, :cw], qrw,
                                         kT_sb[hsl, boff + c * 512:boff + c * 512 + cw],
                                         start=True, stop=True,
                                         tile_position=((hl * d_head, 0)
                                                        if row_tile else None))
                        if skew_accum:
                            if hl == 0:
                                nc.scalar.copy(s2[:, hl, c * 512:c * 512 + cw],
                                               acp[:, :cw])
                            else:
                                nc.vector.tensor_copy(
                                    s2[:, hl, c * 512:c * 512 + cw], acp[:, :cw])
                        else:
                            nc.vector.tensor_tensor(
                                out=s2[:, hl, c * 512:c * 512 + cw],
                                in0=acp[:, :cw],
                                in1=bdsk[:, hl, c * 512:c * 512 + cw],
                                op=ALU.add)
                prob = pbs.tile([P, 2 * qlen], BF16, tag="prob", bufs=2,
                                name="prob")
                rinvs = []
                for hl in range(hpc):
                    if skew_accum:
                        # s2 += rel-shifted BD (carries NEG mask via the pad);
                        # per-head so exp(h0) overlaps the h1 skew
                        nc.gpsimd.dma_start(s2[:, hl, :L],
                                            skew_src_h(pre2, L, hl),
                                            accum_op=ALU.add)
                    rsum = pb.tile([P, 1], F32, tag=f"rsum{hl}", name="rsum")
                    nc.scalar.activation(prob[:, hl * L:(hl + 1) * L],
                                         s2[:, hl, :L], AF.Exp,
                                         scale=scale, accum_out=rsum[:])
                    rinv = pb.tile([P, 1], F32, tag=f"rinv{hl}", name="rinv")
                    nc.vector.reciprocal(rinv[:], rsum[:])
                    rinvs.append(rinv)
                return (I, L, prob, rinvs)

            def stage2b(b, ctx, avT_b):
                """prob transpose + PV + av for row-tile I (one tile behind
                stage2a, so the PE queue never idles on the exp chain)."""
                I, L, prob, rinvs = ctx
                # transpose prob via the PE (DMA transpose would serialize
                # against in-flight collectives), 4 tiles per PSUM evict
                pts = pbs.tile([P, 2 * n_qt, P], BF16, tag="pts", bufs=2,
                               name="pts")
                for hl in range(hpc):
                    for g in range(0, I + 1, 4):
                        gn = min(4, I + 1 - g)
                        tps = pbp.tile([P, 512], BF16, tag="tps", bufs=2,
                                       name="tps")
                        for k in range(gn):
                            nc.tensor.transpose(
                                tps[:, k * P:(k + 1) * P],
                                prob[:, hl * L + (g + k) * P:
                                     hl * L + (g + k + 1) * P],
                                ident_bf[:])
                        dst = pts[:, hl * (I + 1) + g:hl * (I + 1) + g + gn, :]
                        if (g // 4 + hl) % 3 != 0:
                            nc.scalar.copy(dst, tps[:, :gn * P])
                        else:
                            nc.vector.tensor_copy(dst, tps[:, :gn * P])
                pv = pbp.tile([P, hd], F32, tag="pv", bufs=1, name="pv")
                av = pb.tile([P, hd], BF16, tag="av", name="av")
                for hl in range(hpc):
                    hsl = slice(hl * d_head, (hl + 1) * d_head)
                    for J in range(I + 1):
                        nc.tensor.matmul(pv[:, hsl], pts[:, hl * (I + 1) + J, :],
                                         v_sb[:, b * n_qt + J, hsl],
                                         start=(J == 0), stop=(J == I),
                                         skip_group_check=True)
                    nc.vector.tensor_scalar(out=av[:, hsl], in0=pv[:, hsl],
                                            scalar1=rinvs[hl][:], scalar2=None,
                                            op0=ALU.mult)
                tp = pbp.tile([P, 512], BF16, tag="tps", bufs=2, name="avtp")
                nc.tensor.transpose(tp[:, :P], av[:], ident_bf[:])
                nc.scalar.copy(avT_b[:, I * P:(I + 1) * P], tp[:, :P])

            def emit_ln1(b, ppool):
                """residual + LN1 for batch b's owned rows (cc_out[b] ready)."""
                for rt2 in range(tok_own // P):
                    rt = b * (tok_own // P) + rt2
                    rs_bf = pcl.tile([P, d_model], BF16, tag="rsbf")
                    nc.sync.dma_start(rs_bf[:],
                                      cc_out[b][rt2 * P:(rt2 + 1) * P, :])
                    wres = pcl.tile([P, d_model], F32, tag="wres")
                    nc.sync.dma_start(wres[:], w_own[rt * P:(rt + 1) * P, :])
                    nc.vector.tensor_tensor(out=wres[:], in0=wres[:],
                                            in1=rs_bf[:], op=ALU.add)
                    nc.vector.tensor_tensor(out=wres[:], in0=wres[:],
                                            in1=bo_bc[:], op=ALU.add)
                    xn = pcl.tile([P, d_model], F32, tag="xn")
                    layer_norm_xn(wres[:], xn[:], pcl)
                    # transposed normalized copy for the FFN (g folded into W1)
                    for dt in range(n_dmc):
                        xt = ppool.tile([P, P], F32, tag="tpc", bufs=1, name="xt")
                        nc.tensor.transpose(xt[:], xn[:, dt * P:(dt + 1) * P],
                                            ident_f32[:])
                        nc.vector.tensor_copy(
                            x1T_sb[:, dt, rt * P:(rt + 1) * P], xt[:])
                    # full LN1 output for the residual path
                    nc.gpsimd.tensor_tensor(out=xn[:], in0=xn[:], in1=g1_bc[:],
                                            op=ALU.mult)
                    nc.gpsimd.tensor_tensor(out=x1_sb[:, rt, :], in0=xn[:],
                                            in1=b1ln_bc[:], op=ALU.add)

            bdsk_hold = [None]
            for b in range(bsz):
                boff = b * qlen
                avT_b = pb.tile([P, qlen], BF16, tag="avT", bufs=2,
                                name=f"avT{b}")
                carry = stage1(b, 0)
                pend = None
                for I in range(n_qt):
                    nxt = stage1(b, I + 1) if I + 1 < n_qt else None
                    cur = stage2a(b, I, carry)
                    if pend is not None:
                        stage2b(b, pend, avT_b)
                    carry = nxt
                    pend = cur
                stage2b(b, pend, avT_b)
                # o-projection partial for this batch + ReduceScatter
                for T in range(n_qt):
                    wo_bf = pb.tile([P, d_model], BF16, tag="wobf", bufs=2)
                    for gs in range(0, d_model, 512):
                        gw = min(512, d_model - gs)
                        wop = pbp.tile([P, 512], F32, tag="sc", bufs=4)
                        nc.tensor.matmul(wop[:, :gw],
                                         avT_b[:, T * P:(T + 1) * P],
                                         wo_sb[:, gs:gs + gw],
                                         start=True, stop=True)
                        if (T + gs // 512) % 2 == 0:
                            nc.scalar.copy(wo_bf[:, gs:gs + gw], wop[:, :gw])
                        else:
                            nc.vector.tensor_copy(wo_bf[:, gs:gs + gw],
                                                  wop[:, :gw])
                    nc.sync.dma_start(cc_in[b][T * P:(T + 1) * P, :], wo_bf[:])
                nc.gpsimd.collective_compute(
                    "ReduceScatter", ALU.add, replica_groups=rgroups,
                    ins=[cc_in[b][:]], outs=[cc_out[b][:]])
            # phase C: residual + LN1.  The virtual-time floor keeps the
            # scheduler from backfilling these RS-dependent ops into idle
            # slots mid-attention (their sem waits would head-of-line-block
            # the vector/sync queues while a ReduceScatter is in flight).
            with tc.tile_wait_until(50):
                for b in range(2):
                    emit_ln1(b, pbp)
            _b_ctx.close()
            _pbp_ctx.close()
            _ab_ctx.close()

            # ---------------- phase D: FFN + LN2 ----------------
            _d_ctx = ExitStack()
            resD = _d_ctx.enter_context(tc.tile_pool(name="resD", bufs=1))
            hT_sb = resD.tile([P, n_dit, rows_own], BF16)
            b1_sb = resD.tile([P, n_dit], F32)
            nc.sync.dma_start(b1_sb[:], b1_t[:])
            # FFN1 in row-halves: the first half only needs LN1 of batches
            # 0-1, so it streams while RS(3)/LN1(2,3) are still finishing
            with tc.tile_pool(name="pd_w", bufs=3) as pdw, \
                 tc.tile_pool(name="pd_ps", bufs=2, space="PSUM") as pdp:
                for half in range(2):
                    ts = half * 512
                    for dt in range(n_dit):
                        w1c = pdw.tile([P, n_dmc, P], BF16, tag="w1c")
                        nc.sync.dma_start(
                            w1c[:], w1_kd[:, dt * n_dmc * P:(dt + 1) * n_dmc * P])
                        ps = pdp.tile([P, 512], F32, tag="ffn1")
                        for d in range(n_dmc):
                            nc.tensor.matmul(
                                ps[:], w1c[:, d, :],
                                x1T_sb[:, d, ts:ts + 512],
                                start=(d == 0), stop=(d == n_dmc - 1))
                        nc.scalar.activation(
                            hT_sb[:, dt, ts:ts + 512], ps[:],
                            AF.Relu, bias=b1_sb[:, dt:dt + 1])
                    if half == 0:
                        with tc.tile_wait_until(50):
                            for b in range(2, bsz):
                                emit_ln1(b, pdp)

            with tc.tile_pool(name="pe_w", bufs=3) as pew, \
                 tc.tile_pool(name="pe_sb", bufs=3) as pes, \
                 tc.tile_pool(name="pe_ps", bufs=4, space="PSUM") as pep:
                for half in range(2):
                    rts = range(half * (n_rt // 2), (half + 1) * (n_rt // 2))
                    psy = {rt: pep.tile([P, d_model], F32, tag=f"ffn2_{rt % 4}",
                                        name=f"psy{rt}", bufs=1)
                           for rt in rts}
                    for dt in range(n_dit):
                        w2c = pew.tile([P, d_model], BF16, tag="w2c")
                        nc.sync.dma_start(w2c[:], w2_kd[dt * P:(dt + 1) * P, :])
                        for rt in rts:
                            for gs in range(0, d_model, 512):
                                gw = min(512, d_model - gs)
                                nc.tensor.matmul(
                                    psy[rt][:, gs:gs + gw],
                                    hT_sb[:, dt, rt * P:(rt + 1) * P],
                                    w2c[:, gs:gs + gw],
                                    start=(dt == 0), stop=(dt == n_dit - 1))
                    for rt in rts:
                        y_sb = pes.tile([P, d_model], F32, tag="ysb")
                        nc.vector.tensor_tensor(out=y_sb[:], in0=psy[rt][:],
                                                in1=x1_sb[:, rt, :], op=ALU.add)
                        nc.vector.tensor_tensor(out=y_sb[:], in0=y_sb[:],
                                                in1=b2_bc[:], op=ALU.add)
                        o_sb = pes.tile([P, d_model], F32, tag="osb")
                        layer_norm_xn(y_sb[:], o_sb[:], pes)
                        nc.gpsimd.tensor_tensor(out=o_sb[:], in0=o_sb[:],
                                                in1=g2_bc[:], op=ALU.mult)
                        nc.gpsimd.tensor_tensor(out=o_sb[:], in0=o_sb[:],
                                                in1=b2ln_bc[:], op=ALU.add)
                        nc.sync.dma_start(out_own[rt * P:(rt + 1) * P, :], o_sb[:])
            _d_ctx.close()
            _pcl_ctx.close()
    nc.finalize()
    return nc


# ----------------------------------------------------------------------------
# host-side wrapper
# ----------------------------------------------------------------------------
QLEN, BSZ, D_MODEL, N_HEAD, D_HEAD, D_INNER = 2048, 4, 1024, 16, 64, 4096
N_CORES = 8
HPC = N_HEAD // N_CORES

_prog_cache = {}


def _get_program():
    key = "full"
    if key not in _prog_cache:
        _prog_cache[key] = build_program(QLEN, BSZ, D_MODEL, D_HEAD, HPC,
                                         D_INNER, N_CORES)
    return _prog_cache[key]


def make_in_maps(w, r, Wq, bq, Wk, bk, Wv, bv, Wr, br, Wo, bo,
                 r_w_bias, r_r_bias, ln1_g, ln1_b, W1, b1, W2, b2,
                 qlen=QLEN, bsz=BSZ, d_model=D_MODEL, d_head=D_HEAD,
                 hpc=HPC, d_inner=D_INNER, n_cores=N_CORES,
                 ln2_g=None, ln2_b=None):
    f32 = np.float32
    import ml_dtypes
    bf16 = ml_dtypes.bfloat16
    n_dmc = d_model // 128
    wTf = w.transpose(2, 1, 0).reshape(d_model, bsz * qlen)
    wT = np.ascontiguousarray(
        wTf.reshape(n_dmc, 128, (bsz * qlen) // 512, 512)
        .transpose(1, 2, 0, 3).reshape(128, -1)).astype(bf16)
    rT = np.ascontiguousarray(
        np.asarray(r).T.reshape(n_dmc, 128, qlen // 512, 512)
        .transpose(1, 2, 0, 3).reshape(128, -1)).astype(bf16)
    tok_own = qlen // n_cores
    n_dit = d_inner // 128
    # fold LN1 affine into W1/b1 (FFN path consumes the un-affined xn)
    W1f = np.asarray(W1, f32) * np.asarray(ln1_g, f32)[None, :]
    b1f = np.asarray(b1, f32) + np.asarray(W1, f32) @ np.asarray(ln1_b, f32)
    w1_kd = np.ascontiguousarray(
        W1f.T.reshape(n_dmc, 128, n_dit, 128)
        .transpose(1, 2, 0, 3).reshape(128, -1)).astype(bf16)
    w2_kd = np.ascontiguousarray(W2.T).astype(bf16)
    b1_t = np.ascontiguousarray(b1f.reshape(n_dit, 128).T).astype(f32)
    common = dict(
        wT=wT, rT=rT, bo=bo.astype(bf16),
        ln1_g=ln1_g.astype(bf16), ln1_b=ln1_b.astype(bf16),
        ln2_g=ln2_g.astype(bf16), ln2_b=ln2_b.astype(bf16),
        w1_kd=w1_kd, b1_t=b1_t, w2_kd=w2_kd, b2=b2.astype(bf16),
    )
    in_maps = []
    for c in range(n_cores):
        hs = slice(c * hpc * d_head, (c + 1) * hpc * d_head)
        head_sl = slice(c * hpc, (c + 1) * hpc)
        tok_sl = slice(c * tok_own, (c + 1) * tok_own)
        w_own = np.ascontiguousarray(
            w[tok_sl].transpose(1, 0, 2).reshape(bsz * tok_own, d_model)
        ).astype(f32)
        rwb = np.asarray(r_w_bias, f32)[head_sl].reshape(-1)
        rrb = np.asarray(r_r_bias, f32)[head_sl].reshape(-1)
        m = dict(common)
        m.update(
            w_own=w_own,
            wq_t=np.ascontiguousarray(np.asarray(Wq)[hs].T.reshape(n_dmc, 128, 128).transpose(1, 0, 2).reshape(128, -1)).astype(bf16),
            wk_t=np.ascontiguousarray(np.asarray(Wk)[hs].T.reshape(n_dmc, 128, 128).transpose(1, 0, 2).reshape(128, -1)).astype(bf16),
            wv_t=np.ascontiguousarray(np.asarray(Wv)[hs].T.reshape(n_dmc, 128, 128).transpose(1, 0, 2).reshape(128, -1)).astype(bf16),
            wr_t=np.ascontiguousarray(np.asarray(Wr)[hs].T.reshape(n_dmc, 128, 128).transpose(1, 0, 2).reshape(128, -1)).astype(bf16),
            bqrw=(bq[hs].astype(f32) + rwb).astype(f32),
            bqrr=(bq[hs].astype(f32) + rrb).astype(f32),
            bk=bk[hs].astype(f32),
            bv=bv[hs].astype(f32), br=br[hs].astype(f32),
            wo_hd=np.ascontiguousarray(Wo[:, hs].T).astype(bf16),
        )
        in_maps.append(m)
    return in_maps


def assemble_output(results, qlen=QLEN, bsz=BSZ, d_model=D_MODEL,
                    n_cores=N_CORES):
    tok_own = qlen // n_cores
    out = np.empty((qlen, bsz, d_model), np.float32)
    for c in range(n_cores):
        slab = results[c]["out_own"].reshape(bsz, tok_own, d_model)
        out[c * tok_own:(c + 1) * tok_own] = slab.transpose(1, 0, 2)
    return out


def kernel(w, r, attn_mask, Wq, bq, Wk, bk, Wv, bv, Wr, br, Wo, bo,
           r_w_bias, r_r_bias, ln1_g, ln1_b, W1, b1, W2, b2, ln2_g, ln2_b):
    w = np.asarray(w); r = np.asarray(r)
    attn_mask = np.asarray(attn_mask)
    expect = np.triu(np.ones((QLEN, QLEN), dtype=bool), k=1)
    assert np.array_equal(attn_mask, expect), \
        "kernel specializes the causal (triu) attention mask"
    from concourse.bass_utils import run_bass_kernel_spmd
    nc = _get_program()
    in_maps = make_in_maps(np.asarray(w), np.asarray(r),
                           np.asarray(Wq), np.asarray(bq), np.asarray(Wk),
                           np.asarray(bk), np.asarray(Wv), np.asarray(bv),
                           np.asarray(Wr), np.asarray(br), np.asarray(Wo),
                           np.asarray(bo), np.asarray(r_w_bias),
                           np.asarray(r_r_bias), np.asarray(ln1_g),
                           np.asarray(ln1_b), np.asarray(W1), np.asarray(b1),
                           np.asarray(W2), np.asarray(b2),
                           ln2_g=np.asarray(ln2_g), ln2_b=np.asarray(ln2_b))
    res = run_bass_kernel_spmd(nc, in_maps, list(range(N_CORES)))
    return assemble_output(res.results)
